# revision 35
# baseline (speedup 1.0000x reference)
"""Fused attention kernel for nn_Attention_1090921693811, one sample per core.

v3: fp16 x/wqkv (halve input DMA); software-pipelined PE stream (convs run
ahead of depthwise); [128,512] conv PSUM chunks with drains alternating
ACT/DVE; packed head-quadrant gram/softmax mid ([128,96], heads at partition
offsets 0/64) with PE-transpose norm plumbing (no SBUF-SBUF DMAs except
mhatT assembly); tail with 2048-col chunks and full-bandwidth out DMA (the
64-row block packs two column halves onto 128 partitions via a reordered
DRAM access pattern).
"""
import sys
sys.path.insert(0, '/opt/trn_rl_repo')
import numpy as np
from contextlib import ExitStack
from concourse import bass, bacc, mybir, tile

F32 = mybir.dt.float32
FP16 = mybir.dt.float16
Alu = mybir.AluOpType
Act = mybir.ActivationFunctionType

C = 192; C3 = 576; HEADS = 4; CH = 48; H = 128; W = 128; N = H * W
R = 16                   # stripe output rows
NS = H // R              # stripes
SROWS = R + 2            # buffer rows incl halo
STRIDE = 130             # padded row stride: [128 data][2 pad]
ABUF = 2 + (SROWS + 1) * STRIDE
TOPKS = (24, 32, 36, 38)
NEG = -1e30

# o-tiles: T0=q[0:128] T1=q[128:192]+k[0:64] T2=k[64:192] T3=v[0:128] T4=v[128:192]
OT = [(0, 128), (128, 128), (256, 128), (384, 128), (512, 64)]


def host_prep(x, w_qkv, w_dw, w_proj, temperature, attn1, attn2, attn3, attn4):
    x = np.asarray(x, np.float32).reshape(C, N)
    wq = np.asarray(w_qkv, np.float32).reshape(3 * C, C)
    wdw = np.asarray(w_dw, np.float32).reshape(3 * C, 9)
    wp = np.asarray(w_proj, np.float32).reshape(C, C)
    temp = np.asarray(temperature, np.float32).reshape(HEADS)
    wgts = np.stack([np.float32(np.asarray(a).reshape(())) for a in
                     (attn1, attn2, attn3, attn4)])
    d = {"x": x.astype(np.float16),
         "wqkvT": np.ascontiguousarray(wq.T).astype(np.float16),
         "wdw": np.ascontiguousarray(wdw[0:128])}
    for i, (o0, ow) in enumerate(OT):
        dg = np.zeros((ow, 9 * ow), np.float16)
        for t in range(9):
            dg[np.arange(ow), t * ow + np.arange(ow)] = wdw[o0:o0 + ow, t].astype(np.float16)
        d[f"diag{i}"] = dg
    d["ident"] = np.eye(128, dtype=np.float16)
    # T4 pair-packed stationaries: pairs (0,1),(3,4),(6,7) over stacked [A; A<<1]
    dg4p = np.zeros((128, 3 * 64), np.float16)
    dg4s = np.zeros((64, 3 * 64), np.float16)
    for pi, ta in enumerate((0, 3, 6)):
        dg4p[np.arange(64), pi * 64 + np.arange(64)] = wdw[512:576, ta].astype(np.float16)
        dg4p[64 + np.arange(64), pi * 64 + np.arange(64)] = wdw[512:576, ta + 1].astype(np.float16)
    for si, t in enumerate((2, 5, 8)):
        dg4s[np.arange(64), si * 64 + np.arange(64)] = wdw[512:576, t].astype(np.float16)
    d["diag4p"] = dg4p
    d["diag4s"] = dg4s
    # wprojT2: head blocks replicated at partition quadrants 0:48 and 64:112
    wpt = np.zeros((128, HEADS * C), np.float16)
    for h in range(HEADS):
        blk = wp.T[h * CH:(h + 1) * CH, :].astype(np.float16)
        wpt[0:CH, h * C:(h + 1) * C] = blk
        wpt[64:64 + CH, h * C:(h + 1) * C] = blk
    d["wprojT2"] = wpt
    # temp_rep2[p, cb] = temperature[head(2*cb + (p>=64))]
    t2 = np.zeros((128, 2), np.float32)
    t2[0:64, 0] = temp[0]; t2[64:128, 0] = temp[1]
    t2[0:64, 1] = temp[2]; t2[64:128, 1] = temp[3]
    d["temp_rep2"] = t2
    w8 = np.zeros((128, 8), np.float32)
    for cb in range(2):
        w8[:, cb * 4:(cb + 1) * 4] = wgts[None, :]
    d["wgt_rep2"] = w8
    d["ones1"] = np.ones((1, CH), np.float32)
    d["ones_c"] = np.ones((1, 1), np.float32)
    return d


def build(debug=(), reps=1, phase="full"):
    nc = bacc.Bacc("TRN2", target_bir_lowering=False)
    E = {}
    specs = [("x", [C, N], FP16), ("wqkvT", [C, C3], FP16), ("wdw", [128, 9], F32),
             ("ident", [128, 128], FP16),
             ("wprojT2", [128, HEADS * C], FP16),
             ("temp_rep2", [128, 2], F32), ("wgt_rep2", [128, 8], F32),
             ("ones1", [1, CH], F32), ("ones_c", [1, 1], F32)]
    for i, (o0, ow) in enumerate(OT):
        specs.append((f"diag{i}", [ow, 9 * ow], FP16))
    specs += [("diag4p", [128, 3 * 64], FP16), ("diag4s", [64, 3 * 64], FP16)]
    for name, shape, dt in specs:
        E[name] = nc.declare_dram_parameter(name, shape, dt, isOutput=False)
    out_ext = nc.declare_dram_parameter("out", [C, N], F32, isOutput=True)
    dbg_ext = {name: nc.declare_dram_parameter("dbg_" + name, list(shape), F32, isOutput=True)
               for name, shape in debug}
    dbg = dict(debug)

    with tile.TileContext(nc) as tc, ExitStack() as ctx:
        persist = ctx.enter_context(tc.tile_pool(name="persist", bufs=1))
        P = {}
        for name, shape, dt in specs:
            if name in ("x", "wqkvT"):
                continue
            P[name] = persist.tile(shape, dt, tag=name, name=name)
            nc.sync.dma_start(P[name][:], E[name][:])
        P["wq0"] = persist.tile([128, C3], FP16, tag="wq0", name="wq0")
        P["wq1"] = persist.tile([64, C3], FP16, tag="wq1", name="wq1")
        nc.sync.dma_start(P["wq0"][:], E["wqkvT"][0:128, :])
        nc.sync.dma_start(P["wq1"][:], E["wqkvT"][128:192, :])
        mid = ctx.enter_context(tc.tile_pool(name="mid", bufs=1))
        for _rep in range(reps):
            _run_once(nc, tc, mid, persist, P, E, out_ext, dbg_ext,
                      dbg if _rep == reps - 1 else {}, phase)
    nc.finalize()
    return nc


def _run_once(nc, tc, mid, persist, P, E, out_ext, dbg_ext, dbg, phase="full"):
    wq0, wq1 = P["wq0"], P["wq1"]
    diags = [P[f"diag{i}"] for i in range(5)]
    ident = P["ident"]
    wdw_sb = P["wdw"]
    sumsq = persist.tile([128, 3 * NS], F32, tag="ssq", name="ssq")
    v_dw = [persist.tile([128, N], FP16, tag="vdw0", name="vdw0"),
            persist.tile([128, N // 2], FP16, tag="vdw1", name="vdw1")]

    def load_x_stripe(s, pool):
        r0 = max(s * R - 1, 0)
        r1 = min(s * R + R + 1, H)
        br0 = r0 - (s * R - 1)
        nr = r1 - r0
        xs = pool.tile([128, SROWS * W], FP16, tag="xa", name="xa")
        xb = pool.tile([64, SROWS * W], FP16, tag="xb", name="xb")
        if br0 > 0:
            nc.gpsimd.memset(xs[:, 0:W].bitcast(F32), 0.0)
            nc.gpsimd.memset(xb[:, 0:W].bitcast(F32), 0.0)
        if br0 + nr < SROWS:
            nc.gpsimd.memset(xs[:, (SROWS - 1) * W:].bitcast(F32), 0.0)
            nc.gpsimd.memset(xb[:, (SROWS - 1) * W:].bitcast(F32), 0.0)
        nc.sync.dma_start(xs[:, br0 * W:(br0 + nr) * W], E["x"][0:128, r0 * W:r1 * W])
        nc.sync.dma_start(xb[:, br0 * W:(br0 + nr) * W], E["x"][128:192, r0 * W:r1 * W])
        return xs, xb

    # ======================= main stripe loop =======================
    gram_sb = mid.tile([128, 2 * CH], F32, tag="gramsb", name="gramsb")
    with tc.tile_pool(name="p1x", bufs=2) as xp, \
         tc.tile_pool(name="p1gps", bufs=3, space="PSUM") as gps, \
         tc.tile_pool(name="p1ab", bufs=3) as abp, \
         tc.tile_pool(name="p1dw", bufs=3) as dwp, \
         tc.tile_pool(name="p1t", bufs=2) as tp, \
         tc.tile_pool(name="peops", bufs=2, space="PSUM") as pps, \
         tc.tile_pool(name="gramp", bufs=1, space="PSUM") as gram_pool:
        gram_ps = gram_pool.tile([128, 2 * CH], F32, name="gram_ps")
        nc.vector.memset(gram_ps[:], 0.0)
        for s in range(NS):
            xa, xb = load_x_stripe(s, xp)
            qkT = tp.tile([128, R * 384 + 256], FP16, tag="qkT", name="qkT", bufs=2)
            A_t = [None] * 5
            dense_t = [None] * 5

            def conv(i):
                o0, ow = OT[i]
                A = abp.tile([128, ABUF], FP16, tag="A", name="A", bufs=5)
                A_t[i] = A
                nc.gpsimd.memset(A[:, 0:2], 0.0)
                nc.gpsimd.memset(A[:, 2:2 + SROWS * STRIDE].rearrange(
                    "p (r c) -> p r c", c=STRIDE)[:, :, 128:130], 0.0)
                if i == 0:
                    nc.gpsimd.memset(A[:, 2 + SROWS * STRIDE:].bitcast(F32), 0.0)
                ncols = SROWS * W
                eng = 0
                for c0 in range(0, ncols, 512):
                    cw = min(512, ncols - c0)
                    pg = gps.tile([128, 512], F32, tag="g", name="g")
                    nc.tensor.matmul(pg[:ow, 0:cw], wq0[:, o0:o0 + ow],
                                     xa[:, c0:c0 + cw], start=True, stop=False)
                    nc.tensor.matmul(pg[:ow, 0:cw], wq1[:, o0:o0 + ow],
                                     xb[:, c0:c0 + cw], start=False, stop=True)
                    rr, nrow = c0 // W, cw // W
                    dstA = A[:ow, 2 + rr * STRIDE:2 + (rr + nrow) * STRIDE].rearrange(
                        "p (r c) -> p r c", c=STRIDE)[:, :, 0:128]
                    srcP = pg[:ow, 0:nrow * W].rearrange("p (r c) -> p r c", c=W)
                    if eng % 5 < 3:
                        nc.scalar.copy(dstA, srcP)
                    else:
                        nc.vector.tensor_copy(dstA, srcP)
                    if i == 4:
                        dstA2 = A[64:128, 1 + rr * STRIDE:1 + (rr + nrow) * STRIDE].rearrange(
                            "p (r c) -> p r c", c=STRIDE)[:, :, 0:128]
                        if eng % 5 < 3:
                            nc.vector.tensor_copy(dstA2, srcP)
                        else:
                            nc.scalar.copy(dstA2, srcP)
                    eng += 1

            def dve_taps(i):
                # tile-0 depthwise on DVE/ACT: per-tap tensor_scalar mult + add
                o0, ow = OT[i]
                A = A_t[i]
                dense = dwp.tile([128, R * W], FP16, tag="dw", name="dw", bufs=4)
                dense_t[i] = dense
                Bb = abp.tile([128, ABUF], FP16, tag="B", name="B", bufs=2)
                nc.vector.tensor_copy(Bb[:ow, 0:ABUF - 2], A[:ow, 1:ABUF - 1])

                def src_ap(buf, base):
                    return buf[:ow, base:base + R * STRIDE].rearrange(
                        "p (r c) -> p r c", c=STRIDE)[:, :, 0:128]
                d3 = dense[:ow].rearrange("p (r c) -> p r c", c=W)
                nc.vector.tensor_scalar(d3, src_ap(A, 2 + STRIDE), wdw_sb[o0:o0 + ow, 4:5],
                                        None, Alu.mult)
                ntmp = 0
                for dy in (-1, 0, 1):
                    for dx in (-1, 0, 1):
                        if dy == 0 and dx == 0:
                            continue
                        t = (dy + 1) * 3 + (dx + 1)
                        if dx == 0:
                            sap = src_ap(A, 2 + (1 + dy) * STRIDE)
                        else:
                            sap = src_ap(Bb, 2 + (1 + dy) * STRIDE + dx - 1)
                        tmp = dwp.tile([128, R * W], FP16, tag=f"tmp{ntmp % 2}",
                                       name="tmp", bufs=2)
                        ntmp += 1
                        dap = tmp[:ow].rearrange("p (r c) -> p r c", c=W)
                        if t in (0, 6):
                            nc.scalar.activation(dap, sap, Act.Copy, bias=0.0,
                                                 scale=wdw_sb[o0:o0 + ow, t:t + 1])
                        else:
                            nc.vector.tensor_scalar(dap, sap, wdw_sb[o0:o0 + ow, t:t + 1],
                                                    None, Alu.mult)
                        nc.vector.tensor_tensor(dense[:ow], dense[:ow], tmp[:ow], Alu.add)

            def dw_pe(i):
                o0, ow = OT[i]
                A = A_t[i]
                dgt = diags[i]
                dense = (dwp.tile([128, R * W], FP16, tag="dw", name="dw", bufs=4)
                         if i < 3 else None)
                dense_t[i] = dense
                for gi, ch0 in enumerate(range(0, R, 4)):
                    pv = pps.tile([128, 512], F32, tag="pe", name="pe", bufs=2)
                    first = True
                    if i == 4:
                        for pi, ta in enumerate((0, 3, 6)):
                            dy = ta // 3 - 1
                            base = 2 + (1 + ch0 + dy) * STRIDE - 1
                            mov = A[0:128, base:base + 4 * STRIDE].rearrange(
                                "p (r c) -> p r c", c=STRIDE)[:, :, 0:128]
                            nc.tensor.matmul(pv[:ow, :].rearrange("p (r c) -> p r c", c=W),
                                             P["diag4p"][:, pi * 64:(pi + 1) * 64], mov,
                                             start=first, stop=False, skip_group_check=True)
                            first = False
                        for si, t in enumerate((2, 5, 8)):
                            dy = t // 3 - 1
                            base = 2 + (1 + ch0 + dy) * STRIDE + 1
                            mov = A[0:64, base:base + 4 * STRIDE].rearrange(
                                "p (r c) -> p r c", c=STRIDE)[:, :, 0:128]
                            nc.tensor.matmul(pv[:ow, :].rearrange("p (r c) -> p r c", c=W),
                                             P["diag4s"][:, si * 64:(si + 1) * 64], mov,
                                             start=False, stop=(si == 2), skip_group_check=True)
                    else:
                        for dy in (-1, 0, 1):
                            for dx in (-1, 0, 1):
                                t = (dy + 1) * 3 + (dx + 1)
                                base = 2 + (1 + ch0 + dy) * STRIDE + dx
                                mov = A[:ow, base:base + 4 * STRIDE].rearrange(
                                    "p (r c) -> p r c", c=STRIDE)[:, :, 0:128]
                                nc.tensor.matmul(pv[:ow, :].rearrange("p (r c) -> p r c", c=W),
                                                 dgt[:, t * ow:(t + 1) * ow], mov,
                                                 start=first, stop=(t == 8), skip_group_check=True)
                                first = False
                    c0_ = (s * R + ch0) * W
                    if i < 3:
                        if i == 1:
                            nc.scalar.copy(dense[:ow, ch0 * W:(ch0 + 4) * W], pv[:ow, :])
                        else:
                            nc.vector.tensor_copy(dense[:ow, ch0 * W:(ch0 + 4) * W], pv[:ow, :])
                    elif i == 3:
                        if gi % 2 == 0:
                            nc.scalar.copy(v_dw[0][:ow, c0_:c0_ + 4 * W], pv[:ow, :])
                        else:
                            nc.vector.tensor_copy(v_dw[0][:ow, c0_:c0_ + 4 * W], pv[:ow, :])
                    elif c0_ < N // 2:
                        nc.vector.tensor_copy(v_dw[1][0:64, c0_:c0_ + 4 * W], pv[:ow, :])
                    else:
                        nc.vector.tensor_copy(v_dw[1][64:128, c0_ - N // 2:c0_ - N // 2 + 4 * W],
                                              pv[:ow, :])

            def transp(i):
                # q/k tiles: square+accum for sumsq; PE transpose into qkT
                o0, ow = OT[i]
                dense = dense_t[i]
                sq = dwp.tile([128, R * W], FP16, tag="sq", name="sq")
                nc.scalar.activation(sq[:ow], dense[:ow], Act.Square,
                                     accum_out=sumsq[:ow, i * NS + s:i * NS + s + 1])
                for rcg in range(0, R, 8):
                    pt = pps.tile([128, 1024], FP16, tag="pt", name="pt", bufs=2)
                    for j in range(8):
                        nc.tensor.transpose(pt[:, j * 128:j * 128 + ow],
                                            dense[:ow, (rcg + j) * 128:(rcg + j + 1) * 128],
                                            ident[:])
                    off = rcg * 384 + i * 128
                    dst = qkT[:, off:off + 8 * 384].rearrange(
                        "p (r c) -> p r c", c=384)[:, 0:8, 0:ow]
                    nc.vector.tensor_copy(dst, pt[:].rearrange(
                        "p (r c) -> p r c", c=128)[:, :, 0:ow])

            # software-pipelined emission: convs run ahead of depthwise
            conv(0)
            dve_taps(0)
            conv(1)
            conv(2)
            conv(3)
            dw_pe(1)
            dw_pe(2)
            transp(0)
            conv(4)
            dw_pe(3)
            transp(1)
            dw_pe(4)
            transp(2)
            # gram: per-head matmuls into packed quadrants
            # head h: q cols rc*384 + h*48, k cols rc*384 + 192 + h*48
            # out quadrant: [(h%2)*64 : +48, (h//2)*48 : +48]
            for rc in range(R):
                for h in range(HEADS):
                    nc.tensor.matmul(
                        gram_ps[(h % 2) * 64:(h % 2) * 64 + CH,
                                (h // 2) * CH:(h // 2) * CH + CH],
                        qkT[:, rc * 384 + h * CH: rc * 384 + (h + 1) * CH],
                        qkT[:, rc * 384 + 192 + h * CH: rc * 384 + 192 + (h + 1) * CH],
                        start=False, stop=(s == NS - 1 and rc == R - 1),
                        skip_group_check=True)
        nc.vector.tensor_copy(gram_sb[:], gram_ps[:])

    if phase == "stripes":
        nc.sync.dma_start(out_ext[0:128, 0:3 * NS], sumsq[:])
        nc.sync.dma_start(out_ext[0:128, 3 * NS:3 * NS + 2 * CH], gram_sb[:])
        return

    # ======================= MID: norms + topk softmax =======================
    with tc.tile_pool(name="midps", bufs=1, space="PSUM") as mps:
        ssq_col = mid.tile([128, 3], F32, tag="ssqc", name="ssqc")
        for i in range(3):
            nc.vector.tensor_reduce(ssq_col[:, i:i + 1], sumsq[:, i * NS:(i + 1) * NS],
                                    mybir.AxisListType.X, Alu.add)
        # move sumsq columns into q-row / k-row [1, 192] via small DMAs
        # (partition->free vector transposes, as the baseline mid did)
        nrm = mid.tile([1, 2 * C], F32, tag="nrm", name="nrm")
        nc.sync.dma_start(nrm[0:1, 0:128], ssq_col[0:128, 0:1])
        nc.scalar.dma_start(nrm[0:1, 128:192], ssq_col[0:64, 1:2])
        nc.sync.dma_start(nrm[0:1, 192:256], ssq_col[64:128, 1:2])
        nc.scalar.dma_start(nrm[0:1, 256:384], ssq_col[0:128, 2:3])
        nrm2 = mid.tile([1, 2 * C], F32, tag="nrm2", name="nrm2")
        nc.scalar.sqrt(nrm2[:], nrm[:])
        nc.vector.reciprocal(nrm2[:], nrm2[:])
        # s_ps[p, cb] = qr[head(2cb + (p>=64)) * 48 + (p%64)] via tiny matmuls
        s_ps = mps.tile([128, 2], F32, tag="sps", name="sps")
        rk_ps = mps.tile([128, 2 * CH], F32, tag="rkps", name="rkps")
        nc.vector.memset(s_ps[:], 1.0)
        nc.vector.memset(rk_ps[:], 0.0)
        for h in range(HEADS):
            poff = (h % 2) * 64
            cb = h // 2
            nc.tensor.matmul(s_ps[poff:poff + CH, cb:cb + 1],
                             nrm2[0:1, h * CH:(h + 1) * CH], P["ones_c"][:],
                             start=True, stop=True)
            nc.tensor.matmul(rk_ps[poff:poff + CH, cb * CH:(cb + 1) * CH],
                             P["ones1"][:], nrm2[0:1, C + h * CH:C + (h + 1) * CH],
                             start=True, stop=True)
        s_col = mid.tile([128, 2], F32, tag="scol", name="scol")
        nc.vector.tensor_tensor(s_col[:], s_ps[:], P["temp_rep2"][:], Alu.mult)
        attn = mid.tile([128, 2 * CH], F32, tag="attn", name="attn")
        nc.vector.tensor_tensor(attn[:], gram_sb[:], rk_ps[:], Alu.mult)
        srt = mid.tile([128, 2 * 40], F32, tag="srt", name="srt")
        scratch = mid.tile([128, 2 * CH], F32, tag="scr", name="scr")
        e_t = mid.tile([128, 2 * CH], F32, tag="e", name="e")
        acc_m = mid.tile([128, 2 * CH], F32, tag="accm", name="accm")
        mx = mid.tile([128, 2], F32, tag="mx", name="mx")
        sk = mid.tile([128, 8], F32, tag="sk", name="sk")
        cf = mid.tile([128, 8], F32, tag="cf", name="cf")
        junk = mid.tile([128, CH], F32, tag="junk", name="junk")
        nc.vector.tensor_copy(scratch[:], attn[:])
        for cb in range(2):
            ah = attn[:, cb * CH:(cb + 1) * CH]
            sc = scratch[:, cb * CH:(cb + 1) * CH]
            sr = srt[:, cb * 40:(cb + 1) * 40]
            for it in range(5):
                nc.vector.max(sr[:, it * 8:(it + 1) * 8], sc)
                if it < 4:
                    nc.vector.match_replace(sc, sr[:, it * 8:(it + 1) * 8], sc, NEG)
            nc.vector.tensor_scalar(mx[:, cb:cb + 1], sr[:, 0:1], s_col[:, cb:cb + 1],
                                    -1.0, Alu.mult, Alu.mult)
            eh = e_t[:, cb * CH:(cb + 1) * CH]
            nc.scalar.activation(eh, ah, Act.Exp, bias=mx[:, cb:cb + 1],
                                 scale=s_col[:, cb:cb + 1])
            for ki, kk in enumerate(TOPKS):
                th = sr[:, kk - 1:kk]
                nc.vector.scalar_tensor_tensor(junk[:], ah, th, eh, Alu.is_ge, Alu.mult,
                                               accum_out=sk[:, cb * 4 + ki:cb * 4 + ki + 1])
        nc.vector.reciprocal(sk[:], sk[:])
        nc.vector.tensor_tensor(cf[:], sk[:], P["wgt_rep2"][:], Alu.mult)
        for cb in range(2):
            ah = attn[:, cb * CH:(cb + 1) * CH]
            sr = srt[:, cb * 40:(cb + 1) * 40]
            am = acc_m[:, cb * CH:(cb + 1) * CH]
            for ki, kk in enumerate(TOPKS):
                th = sr[:, kk - 1:kk]
                if ki == 0:
                    nc.vector.tensor_scalar(am, ah, th, cf[:, cb * 4:cb * 4 + 1],
                                            Alu.is_ge, Alu.mult)
                else:
                    nc.vector.tensor_scalar(junk[:], ah, th, cf[:, cb * 4 + ki:cb * 4 + ki + 1],
                                            Alu.is_ge, Alu.mult)
                    nc.vector.tensor_tensor(am, am, junk[:], Alu.add)
        nc.vector.tensor_tensor(acc_m[:], acc_m[:], e_t[:], Alu.mult)
        a_bf = mid.tile([128, 2 * CH], FP16, tag="abf", name="abf")
        nc.vector.tensor_copy(a_bf[:], acc_m[:])
        if "attn" in dbg:
            nc.sync.dma_start(dbg_ext["attn"][:], attn[:])
        if "accm" in dbg:
            nc.sync.dma_start(dbg_ext["accm"][:], acc_m[:])
        # mh = a_h^T @ wprojT_h for each head -> [48, 4*192]
        mh_ps = [mps.tile([CH, C], F32, tag=f"mh{h}", name=f"mh{h}")
                 for h in range(HEADS)]
        mh_sb = mid.tile([CH, HEADS * C], FP16, tag="mhsb", name="mhsb")
        for h in range(HEADS):
            poff = (h % 2) * 64
            cb = h // 2
            nc.tensor.matmul(mh_ps[h][:],
                             a_bf[poff:poff + CH, cb * CH:(cb + 1) * CH],
                             P["wprojT2"][poff:poff + CH, h * C:(h + 1) * C],
                             start=True, stop=True)
            nc.vector.tensor_copy(mh_sb[:, h * C:(h + 1) * C], mh_ps[h][:])
    mhatT = [mid.tile([128, C], FP16, tag="mhs0", name="mhs0"),
             mid.tile([128, C], FP16, tag="mhs1", name="mhs1")]
    qdma = [nc.sync, nc.scalar]
    qi = [0]

    def dma_spread(dst, src):
        qdma[qi[0] % len(qdma)].dma_start(dst, src)
        qi[0] += 1
    for h in range(HEADS):
        p0 = h * CH
        if p0 + CH <= 128:
            dma_spread(mhatT[0][p0:p0 + CH, :], mh_sb[:, h * C:(h + 1) * C])
        elif p0 >= 128:
            dma_spread(mhatT[1][p0 - 128:p0 - 128 + CH, :], mh_sb[:, h * C:(h + 1) * C])
            dma_spread(mhatT[1][p0 - 64:p0 - 64 + CH, :], mh_sb[:, h * C:(h + 1) * C])
        else:
            k1 = 128 - p0
            dma_spread(mhatT[0][p0:128, :], mh_sb[0:k1, h * C:(h + 1) * C])
            dma_spread(mhatT[1][0:CH - k1, :], mh_sb[k1:CH, h * C:(h + 1) * C])
            dma_spread(mhatT[1][64:64 + CH - k1, :], mh_sb[k1:CH, h * C:(h + 1) * C])

    if phase == "mid":
        nc.sync.dma_start(out_ext[0:128, 0:C // 2], mhatT[0][:].bitcast(F32))
        nc.sync.dma_start(out_ext[0:128, C // 2:C], mhatT[1][:].bitcast(F32))
        return

    # ======================= tail: out = mhatT.T @ v_dw =======================
    # rows 0:128 as [128, 2048] chunks; rows 128:192 pack two column halves
    # onto 128 partitions and write via a reordered DRAM AP (full DMA width).
    qout = [nc.sync, nc.scalar]
    qo = [0]

    def dma_out(dst, src):
        qout[qo[0] % 2].dma_start(dst, src)
        qo[0] += 1
    with tc.tile_pool(name="p2o", bufs=3) as op, \
         tc.tile_pool(name="p2ops", bufs=2, space="PSUM") as opsA:
        for n0 in range(0, N, 2048):
            po = opsA.tile([128, 2048], F32, tag="poA", name="poA")
            for c0 in range(0, 2048, 512):
                nn0 = n0 + c0
                nc.tensor.matmul(po[:, c0:c0 + 512], mhatT[0][:, 0:128],
                                 v_dw[0][:, nn0:nn0 + 512], start=True, stop=False)
                if nn0 < N // 2:
                    nc.tensor.matmul(po[:, c0:c0 + 512], mhatT[1][0:64, 0:128],
                                     v_dw[1][0:64, nn0:nn0 + 512], start=False, stop=True)
                else:
                    nc.tensor.matmul(po[:, c0:c0 + 512], mhatT[1][64:128, 0:128],
                                     v_dw[1][64:128, nn0 - N // 2:nn0 - N // 2 + 512],
                                     start=False, stop=True)
            ot = op.tile([128, 2048], F32, tag="ot", name="ot")
            if (n0 // 2048) % 2 == 0:
                nc.vector.tensor_copy(ot[:], po[:])
            else:
                nc.scalar.copy(ot[:], po[:])
            dma_out(out_ext[0:128, n0:n0 + 2048], ot[:])
    with tc.tile_pool(name="p2ob", bufs=3) as opb, \
         tc.tile_pool(name="p2opsb", bufs=4, space="PSUM") as opsB:
        for ci in range(8):
            n0 = ci * 1024
            po = opsB.tile([128, 1024], F32, tag="poB", name="poB")
            for half in range(2):
                nn0 = n0 + half * (N // 2)
                pdst = po[half * 64:half * 64 + 64, :]
                for c0 in range(0, 1024, 512):
                    nc.tensor.matmul(pdst[:, c0:c0 + 512], mhatT[0][:, 128:192],
                                     v_dw[0][:, nn0 + c0:nn0 + c0 + 512],
                                     start=True, stop=False)
                    nc.tensor.matmul(pdst[:, c0:c0 + 512],
                                     mhatT[1][half * 64:half * 64 + 64, 128:192],
                                     v_dw[1][half * 64:half * 64 + 64,
                                             n0 + c0:n0 + c0 + 512],
                                     start=False, stop=True)
            ot = opb.tile([128, 1024], F32, tag="ot2", name="ot2")
            if ci % 2 == 0:
                nc.vector.tensor_copy(ot[:], po[:])
            else:
                nc.scalar.copy(ot[:], po[:])
            # rows 128:192: halves packed on partitions 0:64 / 64:128
            dma_out(out_ext[128:192, n0:n0 + 1024], ot[0:64, :])
            dma_out(out_ext[128:192, N // 2 + n0:N // 2 + n0 + 1024], ot[64:128, :])


from concourse.bass_utils import run_bass_kernel_spmd

B = 8
_CACHE = {}


def kernel(**inputs):
    """Full (unsharded) inputs -> full output [8, 192, 128, 128] float32.

    Shards the batch across 8 NeuronCores (one sample per core, pure data
    parallelism), runs the fused Bass kernel SPMD, gathers results.
    """
    x = np.asarray(inputs["x"], np.float32)
    if "nc" not in _CACHE:
        _CACHE["nc"] = build()
    nc = _CACHE["nc"]
    in_maps = [host_prep(x[b], inputs["w_qkv"], inputs["w_dw"], inputs["w_proj"],
                         inputs["temperature"], inputs["attn1"], inputs["attn2"],
                         inputs["attn3"], inputs["attn4"]) for b in range(B)]
    res = run_bass_kernel_spmd(nc, in_maps, list(range(B)))
    out = np.stack([res.results[b]["out"].reshape(C, H, W) for b in range(B)])
    return out.astype(np.float32)


# revision 49
# speedup vs baseline: 9.2672x; 9.2672x over previous
"""Fused attention kernel for nn_Attention_1090921693811, one sample per core.

v3: fp16 x/wqkv (halve input DMA); software-pipelined PE stream (convs run
ahead of depthwise); [128,512] conv PSUM chunks with drains alternating
ACT/DVE; packed head-quadrant gram/softmax mid ([128,96], heads at partition
offsets 0/64) with PE-transpose norm plumbing (no SBUF-SBUF DMAs except
mhatT assembly); tail with 2048-col chunks and full-bandwidth out DMA (the
64-row block packs two column halves onto 128 partitions via a reordered
DRAM access pattern).
"""
import sys
sys.path.insert(0, '/opt/trn_rl_repo')
import numpy as np
from contextlib import ExitStack
from concourse import bass, bacc, mybir, tile

F32 = mybir.dt.float32
FP16 = mybir.dt.float16
Alu = mybir.AluOpType
Act = mybir.ActivationFunctionType

C = 192; C3 = 576; HEADS = 4; CH = 48; H = 128; W = 128; N = H * W
R = 16                   # stripe output rows
NS = H // R              # stripes
SROWS = R + 2            # buffer rows incl halo
STRIDE = 130             # padded row stride: [128 data][2 pad]
ABUF = 2 + (SROWS + 1) * STRIDE
TOPKS = (24, 32, 36, 38)
NEG = -1e30

# o-tiles: T0=q[0:128] T1=q[128:192]+k[0:64] T2=k[64:192] T3=v[0:128] T4=v[128:192]
OT = [(0, 128), (128, 128), (256, 128), (384, 128), (512, 64)]


def host_prep(x, w_qkv, w_dw, w_proj, temperature, attn1, attn2, attn3, attn4):
    x = np.asarray(x, np.float32).reshape(C, N)
    wq = np.asarray(w_qkv, np.float32).reshape(3 * C, C)
    wdw = np.asarray(w_dw, np.float32).reshape(3 * C, 9)
    wp = np.asarray(w_proj, np.float32).reshape(C, C)
    temp = np.asarray(temperature, np.float32).reshape(HEADS)
    wgts = np.stack([np.float32(np.asarray(a).reshape(())) for a in
                     (attn1, attn2, attn3, attn4)])
    # per-o-tile dw weight columns: wdw_t[p, i*9 + t] = wdw[o0_i + p, t]
    wdw_t = np.zeros((128, 45), np.float32)
    for i, (o0, ow) in enumerate(OT):
        wdw_t[0:ow, i * 9:(i + 1) * 9] = wdw[o0:o0 + ow, :]
    d = {"x": x.astype(np.float16),
         "wqkvT": np.ascontiguousarray(wq.T).astype(np.float16),
         "wdw": wdw_t}
    for i, (o0, ow) in enumerate(OT):
        dg = np.zeros((ow, 9 * ow), np.float16)
        for t in range(9):
            dg[np.arange(ow), t * ow + np.arange(ow)] = wdw[o0:o0 + ow, t].astype(np.float16)
        d[f"diag{i}"] = dg
    d["ident"] = np.eye(128, dtype=np.float16)
    # T4 pair-packed stationaries: pairs (0,1),(3,4),(6,7) over stacked [A; A<<1]
    dg4p = np.zeros((128, 3 * 64), np.float16)
    dg4s = np.zeros((64, 3 * 64), np.float16)
    for pi, ta in enumerate((0, 3, 6)):
        dg4p[np.arange(64), pi * 64 + np.arange(64)] = wdw[512:576, ta].astype(np.float16)
        dg4p[64 + np.arange(64), pi * 64 + np.arange(64)] = wdw[512:576, ta + 1].astype(np.float16)
    for si, t in enumerate((2, 5, 8)):
        dg4s[np.arange(64), si * 64 + np.arange(64)] = wdw[512:576, t].astype(np.float16)
    d["diag4p"] = dg4p
    d["diag4s"] = dg4s
    # wprojT2: head blocks replicated at partition quadrants 0:48 and 64:112
    wpt = np.zeros((128, HEADS * C), np.float16)
    for h in range(HEADS):
        blk = wp.T[h * CH:(h + 1) * CH, :].astype(np.float16)
        wpt[0:CH, h * C:(h + 1) * C] = blk
        wpt[64:64 + CH, h * C:(h + 1) * C] = blk
    d["wprojT2"] = wpt
    # temp_rep2[p, cb] = temperature[head(2*cb + (p>=64))]
    t2 = np.zeros((128, 2), np.float32)
    t2[0:64, 0] = temp[0]; t2[64:128, 0] = temp[1]
    t2[0:64, 1] = temp[2]; t2[64:128, 1] = temp[3]
    d["temp_rep2"] = t2
    w8 = np.zeros((128, 8), np.float32)
    for cb in range(2):
        w8[:, cb * 4:(cb + 1) * 4] = wgts[None, :]
    d["wgt_rep2"] = w8
    d["ones1"] = np.ones((1, CH), np.float32)
    d["ones_c"] = np.ones((1, 1), np.float32)
    return d


def build(debug=(), reps=1, phase="full"):
    nc = bacc.Bacc("TRN2", target_bir_lowering=False)
    E = {}
    specs = [("x", [C, N], FP16), ("wqkvT", [C, C3], FP16), ("wdw", [128, 45], F32),
             ("ident", [128, 128], FP16),
             ("wprojT2", [128, HEADS * C], FP16),
             ("temp_rep2", [128, 2], F32), ("wgt_rep2", [128, 8], F32),
             ("ones1", [1, CH], F32), ("ones_c", [1, 1], F32)]
    for i, (o0, ow) in enumerate(OT):
        specs.append((f"diag{i}", [ow, 9 * ow], FP16))
    specs += [("diag4p", [128, 3 * 64], FP16), ("diag4s", [64, 3 * 64], FP16)]
    for name, shape, dt in specs:
        E[name] = nc.declare_dram_parameter(name, shape, dt, isOutput=False)
    out_ext = nc.declare_dram_parameter("out", [C, N], F32, isOutput=True)
    dbg_ext = {name: nc.declare_dram_parameter("dbg_" + name, list(shape), F32, isOutput=True)
               for name, shape in debug}
    dbg = dict(debug)

    with tile.TileContext(nc) as tc, ExitStack() as ctx:
        persist = ctx.enter_context(tc.tile_pool(name="persist", bufs=1))
        P = {}
        # weight loads on the scalar queue so stripe-0 x loads (sync) overlap;
        # wq/diag first: conv(0) and dw(1) need them soonest
        P["wq0"] = persist.tile([128, C3], FP16, tag="wq0", name="wq0")
        P["wq1"] = persist.tile([64, C3], FP16, tag="wq1", name="wq1")
        nc.scalar.dma_start(P["wq0"][:], E["wqkvT"][0:128, :])
        nc.scalar.dma_start(P["wq1"][:], E["wqkvT"][128:192, :])
        early = ["wdw", "diag0", "diag1", "diag2"]
        names = early + [n for n, _, _ in specs
                         if n not in ("x", "wqkvT") and n not in early]
        byname = {n: (sh, dt) for n, sh, dt in specs}
        for name in names:
            sh, dt = byname[name]
            P[name] = persist.tile(sh, dt, tag=name, name=name)
            nc.scalar.dma_start(P[name][:], E[name][:])
        mid = ctx.enter_context(tc.tile_pool(name="mid", bufs=1))
        for _rep in range(reps):
            _run_once(nc, tc, mid, persist, P, E, out_ext, dbg_ext,
                      dbg if _rep == reps - 1 else {}, phase)
    nc.finalize()
    return nc


def _run_once(nc, tc, mid, persist, P, E, out_ext, dbg_ext, dbg, phase="full"):
    wq0, wq1 = P["wq0"], P["wq1"]
    diags = [P[f"diag{i}"] for i in range(5)]
    ident = P["ident"]
    wdw_sb = P["wdw"]
    sumsq = persist.tile([128, 3 * NS], F32, tag="ssq", name="ssq")
    v_dw = [persist.tile([128, N], FP16, tag="vdw0", name="vdw0"),
            persist.tile([128, N // 2], FP16, tag="vdw1", name="vdw1")]

    def load_x_stripe(s, pool):
        r0 = max(s * R - 1, 0)
        r1 = min(s * R + R + 1, H)
        br0 = r0 - (s * R - 1)
        nr = r1 - r0
        xs = pool.tile([128, SROWS * W], FP16, tag="xa", name="xa")
        xb = pool.tile([64, SROWS * W], FP16, tag="xb", name="xb")
        if br0 > 0:
            nc.gpsimd.memset(xs[:, 0:W].bitcast(F32), 0.0)
            nc.gpsimd.memset(xb[:, 0:W].bitcast(F32), 0.0)
        if br0 + nr < SROWS:
            nc.gpsimd.memset(xs[:, (SROWS - 1) * W:].bitcast(F32), 0.0)
            nc.gpsimd.memset(xb[:, (SROWS - 1) * W:].bitcast(F32), 0.0)
        nc.sync.dma_start(xs[:, br0 * W:(br0 + nr) * W], E["x"][0:128, r0 * W:r1 * W])
        nc.sync.dma_start(xb[:, br0 * W:(br0 + nr) * W], E["x"][128:192, r0 * W:r1 * W])
        return xs, xb

    # ======================= main stripe loop =======================
    gram_sb = mid.tile([128, 2 * CH], F32, tag="gramsb", name="gramsb")
    with tc.tile_pool(name="p1x", bufs=2) as xp, \
         tc.tile_pool(name="p1gps", bufs=3, space="PSUM") as gps, \
         tc.tile_pool(name="p1ab", bufs=3) as abp, \
         tc.tile_pool(name="p1dw", bufs=3) as dwp, \
         tc.tile_pool(name="p1t", bufs=2) as tp, \
         tc.tile_pool(name="peops", bufs=2, space="PSUM") as pps, \
         tc.tile_pool(name="gramp", bufs=1, space="PSUM") as gram_pool:
        gram_ps = gram_pool.tile([128, 2 * CH], F32, name="gram_ps")
        nc.vector.memset(gram_ps[:], 0.0)
        for s in range(NS):
            xa, xb = load_x_stripe(s, xp)
            qkT = tp.tile([128, R * 384 + 256], FP16, tag="qkT", name="qkT", bufs=1)
            A_t = [None] * 5
            dense_t = [None] * 5

            def conv(i):
                o0, ow = OT[i]
                A = abp.tile([128, ABUF], FP16, tag="A", name="A", bufs=4)
                A_t[i] = A
                nc.gpsimd.memset(A[:, 0:2], 0.0)
                nc.gpsimd.memset(A[:, 2:2 + SROWS * STRIDE].rearrange(
                    "p (r c) -> p r c", c=STRIDE)[:, :, 128:130], 0.0)
                if i == 0:
                    nc.gpsimd.memset(A[:, 2 + SROWS * STRIDE:].bitcast(F32), 0.0)
                ncols = SROWS * W
                eng = 0
                for c0 in range(0, ncols, 512):
                    cw = min(512, ncols - c0)
                    pg = gps.tile([128, 512], F32, tag="g", name="g")
                    nc.tensor.matmul(pg[:ow, 0:cw], wq0[:, o0:o0 + ow],
                                     xa[:, c0:c0 + cw], start=True, stop=False)
                    nc.tensor.matmul(pg[:ow, 0:cw], wq1[:, o0:o0 + ow],
                                     xb[:, c0:c0 + cw], start=False, stop=True)
                    rr, nrow = c0 // W, cw // W
                    dstA = A[:ow, 2 + rr * STRIDE:2 + (rr + nrow) * STRIDE].rearrange(
                        "p (r c) -> p r c", c=STRIDE)[:, :, 0:128]
                    srcP = pg[:ow, 0:nrow * W].rearrange("p (r c) -> p r c", c=W)
                    nc.scalar.copy(dstA, srcP)
                    if i == 4:
                        dstA2 = A[64:128, 1 + rr * STRIDE:1 + (rr + nrow) * STRIDE].rearrange(
                            "p (r c) -> p r c", c=STRIDE)[:, :, 0:128]
                        nc.vector.tensor_copy(dstA2, srcP)
                    eng += 1

            def dve_taps(i):
                # tile-0 depthwise on DVE/ACT: per-tap tensor_scalar mult + add
                o0, ow = OT[i]
                A = A_t[i]
                dense = dwp.tile([128, R * W], FP16, tag="dw", name="dw", bufs=4)
                dense_t[i] = dense
                Bb = abp.tile([128, ABUF], FP16, tag="B", name="B", bufs=2)
                nc.vector.tensor_copy(Bb[:ow, 0:ABUF - 2], A[:ow, 1:ABUF - 1])

                def src_ap(buf, base):
                    return buf[:ow, base:base + R * STRIDE].rearrange(
                        "p (r c) -> p r c", c=STRIDE)[:, :, 0:128]
                d3 = dense[:ow].rearrange("p (r c) -> p r c", c=W)
                nc.vector.tensor_scalar(d3, src_ap(A, 2 + STRIDE),
                                        wdw_sb[0:ow, i * 9 + 4:i * 9 + 5],
                                        None, Alu.mult)
                ntmp = 0
                for dy in (-1, 0, 1):
                    for dx in (-1, 0, 1):
                        if dy == 0 and dx == 0:
                            continue
                        t = (dy + 1) * 3 + (dx + 1)
                        if dx == 0:
                            sap = src_ap(A, 2 + (1 + dy) * STRIDE)
                        else:
                            sap = src_ap(Bb, 2 + (1 + dy) * STRIDE + dx - 1)
                        tmp = dwp.tile([128, R * W], FP16, tag=f"tmp{ntmp % 2}",
                                       name="tmp", bufs=2)
                        ntmp += 1
                        dap = tmp[:ow].rearrange("p (r c) -> p r c", c=W)
                        if t in (0, 6):
                            nc.scalar.activation(dap, sap, Act.Copy, bias=0.0,
                                                 scale=wdw_sb[0:ow, i * 9 + t:i * 9 + t + 1])
                        else:
                            nc.vector.tensor_scalar(dap, sap,
                                                    wdw_sb[0:ow, i * 9 + t:i * 9 + t + 1],
                                                    None, Alu.mult)
                        nc.vector.tensor_tensor(dense[:ow], dense[:ow], tmp[:ow], Alu.add)

            part_t = [None] * 5

            def dve_taps_dx0(i):
                # dx==0 taps (t=1,4,7) on DVE into a partial; PE does the rest
                o0, ow = OT[i]
                A = A_t[i]
                part = dwp.tile([128, R * W], FP16, tag="part", name="part", bufs=2)
                part_t[i] = part

                def src_ap(base):
                    return A[:ow, base:base + R * STRIDE].rearrange(
                        "p (r c) -> p r c", c=STRIDE)[:, :, 0:128]
                p3 = part[:ow].rearrange("p (r c) -> p r c", c=W)
                nc.vector.tensor_scalar(p3, src_ap(2 + STRIDE),
                                        wdw_sb[0:ow, i * 9 + 4:i * 9 + 5],
                                        None, Alu.mult)
                for dy in (-1, 1):
                    t = (dy + 1) * 3 + 1
                    tmp = dwp.tile([128, R * W], FP16, tag=f"tmp{dy > 0}",
                                   name="tmp", bufs=2)
                    nc.vector.tensor_scalar(tmp[:ow].rearrange("p (r c) -> p r c", c=W),
                                            src_ap(2 + (1 + dy) * STRIDE),
                                            wdw_sb[0:ow, i * 9 + t:i * 9 + t + 1],
                                            None, Alu.mult)
                    nc.vector.tensor_tensor(part[:ow], part[:ow], tmp[:ow], Alu.add)

            def dw_pe(i, skip_dx0=False):
                o0, ow = OT[i]
                A = A_t[i]
                dgt = diags[i]
                dense = (dwp.tile([128, R * W], FP16, tag="dw", name="dw", bufs=4)
                         if i < 3 else None)
                dense_t[i] = dense
                for gi, ch0 in enumerate(range(0, R, 4)):
                    pv = pps.tile([128, 512], F32, tag="pe", name="pe", bufs=2)
                    first = True
                    if i == 4:
                        for pi, ta in enumerate((0, 3, 6)):
                            dy = ta // 3 - 1
                            base = 2 + (1 + ch0 + dy) * STRIDE - 1
                            mov = A[0:128, base:base + 4 * STRIDE].rearrange(
                                "p (r c) -> p r c", c=STRIDE)[:, :, 0:128]
                            nc.tensor.matmul(pv[:ow, :].rearrange("p (r c) -> p r c", c=W),
                                             P["diag4p"][:, pi * 64:(pi + 1) * 64], mov,
                                             start=first, stop=False, skip_group_check=True)
                            first = False
                        for si, t in enumerate((2, 5, 8)):
                            dy = t // 3 - 1
                            base = 2 + (1 + ch0 + dy) * STRIDE + 1
                            mov = A[0:64, base:base + 4 * STRIDE].rearrange(
                                "p (r c) -> p r c", c=STRIDE)[:, :, 0:128]
                            nc.tensor.matmul(pv[:ow, :].rearrange("p (r c) -> p r c", c=W),
                                             P["diag4s"][:, si * 64:(si + 1) * 64], mov,
                                             start=False, stop=(si == 2), skip_group_check=True)
                    else:
                        for dy in (-1, 0, 1):
                            for dx in (-1, 0, 1):
                                if skip_dx0 and dx == 0:
                                    continue
                                t = (dy + 1) * 3 + (dx + 1)
                                base = 2 + (1 + ch0 + dy) * STRIDE + dx
                                mov = A[:ow, base:base + 4 * STRIDE].rearrange(
                                    "p (r c) -> p r c", c=STRIDE)[:, :, 0:128]
                                nc.tensor.matmul(pv[:ow, :].rearrange("p (r c) -> p r c", c=W),
                                                 dgt[:, t * ow:(t + 1) * ow], mov,
                                                 start=first, stop=(t == 8), skip_group_check=True)
                                first = False
                    c0_ = (s * R + ch0) * W
                    if i < 3:
                        if i == 1:
                            nc.vector.tensor_copy(dense[:ow, ch0 * W:(ch0 + 4) * W], pv[:ow, :])
                        else:
                            nc.scalar.copy(dense[:ow, ch0 * W:(ch0 + 4) * W], pv[:ow, :])
                    elif i == 3:
                        part = part_t[i]
                        if part is not None:
                            nc.vector.tensor_tensor(v_dw[0][:ow, c0_:c0_ + 4 * W],
                                                    pv[:ow, :],
                                                    part[:ow, ch0 * W:(ch0 + 4) * W],
                                                    Alu.add)
                        elif gi % 2 == 0:
                            nc.scalar.copy(v_dw[0][:ow, c0_:c0_ + 4 * W], pv[:ow, :])
                        else:
                            nc.vector.tensor_copy(v_dw[0][:ow, c0_:c0_ + 4 * W], pv[:ow, :])
                    elif c0_ < N // 2:
                        nc.scalar.copy(v_dw[1][0:64, c0_:c0_ + 4 * W], pv[:ow, :])
                    else:
                        nc.scalar.copy(v_dw[1][64:128, c0_ - N // 2:c0_ - N // 2 + 4 * W],
                                       pv[:ow, :])

            def transp(i):
                # q/k tiles: square+accum for sumsq; PE transpose into qkT
                o0, ow = OT[i]
                dense = dense_t[i]
                sq = dwp.tile([128, R * W], FP16, tag="sq", name="sq")
                nc.scalar.activation(sq[:ow], dense[:ow], Act.Square,
                                     accum_out=sumsq[:ow, i * NS + s:i * NS + s + 1])
                for rcg in range(0, R, 8):
                    pt = pps.tile([128, 1024], FP16, tag="pt", name="pt", bufs=2)
                    for j in range(8):
                        nc.tensor.transpose(pt[:, j * 128:j * 128 + ow],
                                            dense[:ow, (rcg + j) * 128:(rcg + j + 1) * 128],
                                            ident[:])
                    off = rcg * 384 + i * 128
                    dst = qkT[:, off:off + 8 * 384].rearrange(
                        "p (r c) -> p r c", c=384)[:, 0:8, 0:ow]
                    nc.vector.tensor_copy(dst, pt[:].rearrange(
                        "p (r c) -> p r c", c=128)[:, :, 0:ow])

            # software-pipelined emission: convs run ahead of depthwise
            conv(0)
            dve_taps(0)
            conv(1)
            conv(2)
            conv(3)
            dw_pe(1)
            dve_taps_dx0(3)
            dw_pe(2)
            transp(0)
            conv(4)
            dw_pe(3, skip_dx0=True)
            transp(1)
            dw_pe(4)
            transp(2)
            # gram: per-head matmuls into packed quadrants
            # head h: q cols rc*384 + h*48, k cols rc*384 + 192 + h*48
            # out quadrant: [(h%2)*64 : +48, (h//2)*48 : +48]
            for rc in range(R):
                for h in range(HEADS):
                    nc.tensor.matmul(
                        gram_ps[(h % 2) * 64:(h % 2) * 64 + CH,
                                (h // 2) * CH:(h // 2) * CH + CH],
                        qkT[:, rc * 384 + h * CH: rc * 384 + (h + 1) * CH],
                        qkT[:, rc * 384 + 192 + h * CH: rc * 384 + 192 + (h + 1) * CH],
                        start=False, stop=(s == NS - 1 and rc == R - 1),
                        skip_group_check=True)
        nc.vector.tensor_copy(gram_sb[:], gram_ps[:])

    if phase == "stripes":
        nc.sync.dma_start(out_ext[0:128, 0:3 * NS], sumsq[:])
        nc.sync.dma_start(out_ext[0:128, 3 * NS:3 * NS + 2 * CH], gram_sb[:])
        return

    # ======================= MID: norms + topk softmax =======================
    with tc.tile_pool(name="midps", bufs=1, space="PSUM") as mps:
        ssq_col = mid.tile([128, 3], F32, tag="ssqc", name="ssqc")
        for i in range(3):
            nc.vector.tensor_reduce(ssq_col[:, i:i + 1], sumsq[:, i * NS:(i + 1) * NS],
                                    mybir.AxisListType.X, Alu.add)
        # move sumsq columns into q-row / k-row [1, 192] via small DMAs
        # (partition->free vector transposes, as the baseline mid did)
        nrm = mid.tile([1, 2 * C], F32, tag="nrm", name="nrm")
        nc.sync.dma_start(nrm[0:1, 0:128], ssq_col[0:128, 0:1])
        nc.scalar.dma_start(nrm[0:1, 128:192], ssq_col[0:64, 1:2])
        nc.sync.dma_start(nrm[0:1, 192:256], ssq_col[64:128, 1:2])
        nc.scalar.dma_start(nrm[0:1, 256:384], ssq_col[0:128, 2:3])
        nrm2 = mid.tile([1, 2 * C], F32, tag="nrm2", name="nrm2")
        nc.scalar.sqrt(nrm2[:], nrm[:])
        nc.vector.reciprocal(nrm2[:], nrm2[:])
        # s_ps[p, cb] = qr[head(2cb + (p>=64)) * 48 + (p%64)] via tiny matmuls
        s_ps = mps.tile([128, 2], F32, tag="sps", name="sps")
        rk_ps = mps.tile([128, 2 * CH], F32, tag="rkps", name="rkps")
        nc.vector.memset(s_ps[:], 1.0)
        nc.vector.memset(rk_ps[:], 0.0)
        for h in range(HEADS):
            poff = (h % 2) * 64
            cb = h // 2
            nc.tensor.matmul(s_ps[poff:poff + CH, cb:cb + 1],
                             nrm2[0:1, h * CH:(h + 1) * CH], P["ones_c"][:],
                             start=True, stop=True)
            nc.tensor.matmul(rk_ps[poff:poff + CH, cb * CH:(cb + 1) * CH],
                             P["ones1"][:], nrm2[0:1, C + h * CH:C + (h + 1) * CH],
                             start=True, stop=True)
        s_col = mid.tile([128, 2], F32, tag="scol", name="scol")
        nc.vector.tensor_tensor(s_col[:], s_ps[:], P["temp_rep2"][:], Alu.mult)
        attn = mid.tile([128, 2 * CH], F32, tag="attn", name="attn")
        nc.vector.tensor_tensor(attn[:], gram_sb[:], rk_ps[:], Alu.mult)
        srt = mid.tile([128, 2 * 40], F32, tag="srt", name="srt")
        scratch = mid.tile([128, 2 * CH], F32, tag="scr", name="scr")
        e_t = mid.tile([128, 2 * CH], F32, tag="e", name="e")
        acc_m = mid.tile([128, 2 * CH], F32, tag="accm", name="accm")
        mx = mid.tile([128, 2], F32, tag="mx", name="mx")
        sk = mid.tile([128, 8], F32, tag="sk", name="sk")
        cf = mid.tile([128, 8], F32, tag="cf", name="cf")
        junk = mid.tile([128, CH], F32, tag="junk", name="junk")
        nc.vector.tensor_copy(scratch[:], attn[:])
        for cb in range(2):
            ah = attn[:, cb * CH:(cb + 1) * CH]
            sc = scratch[:, cb * CH:(cb + 1) * CH]
            sr = srt[:, cb * 40:(cb + 1) * 40]
            for it in range(5):
                nc.vector.max(sr[:, it * 8:(it + 1) * 8], sc)
                if it < 4:
                    nc.vector.match_replace(sc, sr[:, it * 8:(it + 1) * 8], sc, NEG)
            nc.vector.tensor_scalar(mx[:, cb:cb + 1], sr[:, 0:1], s_col[:, cb:cb + 1],
                                    -1.0, Alu.mult, Alu.mult)
            eh = e_t[:, cb * CH:(cb + 1) * CH]
            nc.scalar.activation(eh, ah, Act.Exp, bias=mx[:, cb:cb + 1],
                                 scale=s_col[:, cb:cb + 1])
            for ki, kk in enumerate(TOPKS):
                th = sr[:, kk - 1:kk]
                nc.vector.scalar_tensor_tensor(junk[:], ah, th, eh, Alu.is_ge, Alu.mult,
                                               accum_out=sk[:, cb * 4 + ki:cb * 4 + ki + 1])
        nc.vector.reciprocal(sk[:], sk[:])
        nc.vector.tensor_tensor(cf[:], sk[:], P["wgt_rep2"][:], Alu.mult)
        for cb in range(2):
            ah = attn[:, cb * CH:(cb + 1) * CH]
            sr = srt[:, cb * 40:(cb + 1) * 40]
            am = acc_m[:, cb * CH:(cb + 1) * CH]
            for ki, kk in enumerate(TOPKS):
                th = sr[:, kk - 1:kk]
                if ki == 0:
                    nc.vector.tensor_scalar(am, ah, th, cf[:, cb * 4:cb * 4 + 1],
                                            Alu.is_ge, Alu.mult)
                else:
                    nc.vector.tensor_scalar(junk[:], ah, th, cf[:, cb * 4 + ki:cb * 4 + ki + 1],
                                            Alu.is_ge, Alu.mult)
                    nc.vector.tensor_tensor(am, am, junk[:], Alu.add)
        nc.vector.tensor_tensor(acc_m[:], acc_m[:], e_t[:], Alu.mult)
        a_bf = mid.tile([128, 2 * CH], FP16, tag="abf", name="abf")
        nc.vector.tensor_copy(a_bf[:], acc_m[:])
        if "attn" in dbg:
            nc.sync.dma_start(dbg_ext["attn"][:], attn[:])
        if "accm" in dbg:
            nc.sync.dma_start(dbg_ext["accm"][:], acc_m[:])
        # mh = a_h^T @ wprojT_h for each head -> [48, 4*192]
        mh_ps = [mps.tile([CH, C], F32, tag=f"mh{h}", name=f"mh{h}")
                 for h in range(HEADS)]
        mh_sb = mid.tile([CH, HEADS * C], FP16, tag="mhsb", name="mhsb")
        for h in range(HEADS):
            poff = (h % 2) * 64
            cb = h // 2
            nc.tensor.matmul(mh_ps[h][:],
                             a_bf[poff:poff + CH, cb * CH:(cb + 1) * CH],
                             P["wprojT2"][poff:poff + CH, h * C:(h + 1) * C],
                             start=True, stop=True)
            nc.vector.tensor_copy(mh_sb[:, h * C:(h + 1) * C], mh_ps[h][:])
    mhatT = [mid.tile([128, C], FP16, tag="mhs0", name="mhs0"),
             mid.tile([128, C], FP16, tag="mhs1", name="mhs1")]
    qdma = [nc.sync, nc.scalar]
    qi = [0]

    def dma_spread(dst, src):
        qdma[qi[0] % len(qdma)].dma_start(dst, src)
        qi[0] += 1
    for h in range(HEADS):
        p0 = h * CH
        if p0 + CH <= 128:
            dma_spread(mhatT[0][p0:p0 + CH, :], mh_sb[:, h * C:(h + 1) * C])
        elif p0 >= 128:
            dma_spread(mhatT[1][p0 - 128:p0 - 128 + CH, :], mh_sb[:, h * C:(h + 1) * C])
            dma_spread(mhatT[1][p0 - 64:p0 - 64 + CH, :], mh_sb[:, h * C:(h + 1) * C])
        else:
            k1 = 128 - p0
            dma_spread(mhatT[0][p0:128, :], mh_sb[0:k1, h * C:(h + 1) * C])
            dma_spread(mhatT[1][0:CH - k1, :], mh_sb[k1:CH, h * C:(h + 1) * C])
            dma_spread(mhatT[1][64:64 + CH - k1, :], mh_sb[k1:CH, h * C:(h + 1) * C])

    if phase == "mid":
        nc.sync.dma_start(out_ext[0:128, 0:C // 2], mhatT[0][:].bitcast(F32))
        nc.sync.dma_start(out_ext[0:128, C // 2:C], mhatT[1][:].bitcast(F32))
        return

    # ======================= tail: out = mhatT.T @ v_dw =======================
    # rows 0:128 as [128, 2048] chunks; rows 128:192 pack two column halves
    # onto 128 partitions and write via a reordered DRAM AP (full DMA width).
    qout = [nc.sync, nc.scalar]
    qo = [0]

    def dma_out(dst, src):
        qout[qo[0] % 2].dma_start(dst, src)
        qo[0] += 1
    with tc.tile_pool(name="p2o", bufs=3) as op, \
         tc.tile_pool(name="p2ops", bufs=2, space="PSUM") as opsA:
        for n0 in range(0, N, 2048):
            po = opsA.tile([128, 2048], F32, tag="poA", name="poA")
            for c0 in range(0, 2048, 512):
                nn0 = n0 + c0
                nc.tensor.matmul(po[:, c0:c0 + 512], mhatT[0][:, 0:128],
                                 v_dw[0][:, nn0:nn0 + 512], start=True, stop=False)
                if nn0 < N // 2:
                    nc.tensor.matmul(po[:, c0:c0 + 512], mhatT[1][0:64, 0:128],
                                     v_dw[1][0:64, nn0:nn0 + 512], start=False, stop=True)
                else:
                    nc.tensor.matmul(po[:, c0:c0 + 512], mhatT[1][64:128, 0:128],
                                     v_dw[1][64:128, nn0 - N // 2:nn0 - N // 2 + 512],
                                     start=False, stop=True)
            ot = op.tile([128, 2048], F32, tag="ot", name="ot")
            if (n0 // 2048) % 2 == 0:
                nc.vector.tensor_copy(ot[:], po[:])
            else:
                nc.scalar.copy(ot[:], po[:])
            dma_out(out_ext[0:128, n0:n0 + 2048], ot[:])
    with tc.tile_pool(name="p2ob", bufs=3) as opb, \
         tc.tile_pool(name="p2opsb", bufs=2, space="PSUM") as opsB:
        for ci in range(4):
            n0 = ci * 2048
            po = opsB.tile([128, 2048], F32, tag="poB", name="poB")
            for half in range(2):
                nn0 = n0 + half * (N // 2)
                pdst = po[half * 64:half * 64 + 64, :]
                for c0 in range(0, 2048, 512):
                    nc.tensor.matmul(pdst[:, c0:c0 + 512], mhatT[0][:, 128:192],
                                     v_dw[0][:, nn0 + c0:nn0 + c0 + 512],
                                     start=True, stop=False)
                    nc.tensor.matmul(pdst[:, c0:c0 + 512],
                                     mhatT[1][half * 64:half * 64 + 64, 128:192],
                                     v_dw[1][half * 64:half * 64 + 64,
                                             n0 + c0:n0 + c0 + 512],
                                     start=False, stop=True)
            ot = opb.tile([128, 2048], F32, tag="ot2", name="ot2")
            if ci % 2 == 0:
                nc.vector.tensor_copy(ot[:], po[:])
            else:
                nc.scalar.copy(ot[:], po[:])
            # rows 128:192: halves packed on partitions 0:64 / 64:128
            dma_out(out_ext[128:192, n0:n0 + 2048], ot[0:64, :])
            dma_out(out_ext[128:192, N // 2 + n0:N // 2 + n0 + 2048], ot[64:128, :])


from concourse.bass_utils import run_bass_kernel_spmd

B = 8
_CACHE = {}


def kernel(**inputs):
    """Full (unsharded) inputs -> full output [8, 192, 128, 128] float32.

    Shards the batch across 8 NeuronCores (one sample per core, pure data
    parallelism), runs the fused Bass kernel SPMD, gathers results.
    """
    x = np.asarray(inputs["x"], np.float32)
    if "nc" not in _CACHE:
        _CACHE["nc"] = build()
    nc = _CACHE["nc"]
    in_maps = [host_prep(x[b], inputs["w_qkv"], inputs["w_dw"], inputs["w_proj"],
                         inputs["temperature"], inputs["attn1"], inputs["attn2"],
                         inputs["attn3"], inputs["attn4"]) for b in range(B)]
    res = run_bass_kernel_spmd(nc, in_maps, list(range(B)))
    out = np.stack([res.results[b]["out"].reshape(C, H, W) for b in range(B)])
    return out.astype(np.float32)
